# revision 6
# baseline (speedup 1.0000x reference)
"""MiniCPM attention block on 8 Trainium2 NeuronCores.

Sharding: core c handles batch b = c // 4 and the 8 heads
[ (c%4)*8, (c%4)*8 + 8 ) of that batch (tensor-parallel over heads +
data-parallel over batch).  Each core computes a partial output
x @ block-of-Wo.T of shape [S, HID]; the host sums the 4 partials per
batch.  No collectives.

Device pipeline per core (S=2048 tokens, 8 heads of d=64):
  1a. qT/kT = (x @ Wq_s.T).T with RoPE fused into the PSUM->SBUF evict
  1b. v     =  x @ Wv_s.T, stored [tk, 8*65] with a ones column per head
  2.  per head-pair, per tq-chunk j (512): S.T tiles [tk 128, tq 512] on
      PE (2 heads packed via row tiling), exp on ACT, causal zeroing via
      gpsimd affine_select, AV accumulation on PE with the ones column
      producing the softmax denominator for free; normalize with
      DVE reciprocal + DMA partition-broadcast + DVE multiply.
  3.  out_partial = attn_outT.T @ Wo_s.T chunks on PE.

Everything is self-contained: shapes hardcoded from the problem spec.
"""
import numpy as np
import ml_dtypes

S = 2048
HID = 2048
NH = 32
DH = 64
N_CORES = 8
HEADS_PER_CORE = NH // (N_CORES // 2)   # 8
BLK = HEADS_PER_CORE * DH               # 512
ROPE_BASE = 10000.0

_PROGRAMS = {}


def _rope_cache():
    inv_freq = 1.0 / (ROPE_BASE ** (np.arange(0, DH, 2, dtype=np.float32) / DH))
    t = np.arange(S, dtype=np.float32)
    freqs = np.outer(t, inv_freq)                     # [S, 32]
    emb = np.concatenate([freqs, freqs], axis=-1)     # [S, 64]
    return np.cos(emb), np.sin(emb)


def build_program(variant="causal", dtype="bf16"):
    """Build the Bacc program (one NEFF, run SPMD on 8 cores)."""
    import concourse.bass as bass
    import concourse.mybir as mybir
    import concourse.tile as tile
    from concourse import bacc

    fp32 = mybir.dt.float32
    if dtype == "bf16":
        DT = mybir.dt.bfloat16      # dram/lhs/rhs matmul dtype
        SDT = mybir.dt.bfloat16     # on-chip storage for q/k/v/p/attn
        CH = 512                    # token chunk for phase 1
        MDT = DT                    # matmul ap dtype (no bitcast needed)
    elif dtype == "fp32r":
        DT = mybir.dt.float32
        SDT = mybir.dt.float32
        CH = 256
        MDT = mybir.dt.float32r
    else:
        raise ValueError(dtype)

    def mm(ap):
        return ap.bitcast(MDT) if MDT is not ap.dtype else ap

    causal = variant == "causal"
    NCH = S // CH            # phase-1 token chunks
    NT = S // 128            # 16 token tiles
    NPR = 4                  # head pairs

    nc = bacc.Bacc("TRN2", target_bir_lowering=False, debug=False,
                   enable_asserts=False, num_devices=N_CORES)

    xT = nc.dram_tensor("xT", [HID, S], DT, kind="ExternalInput").ap()
    wqT = nc.dram_tensor("wqT", [HID, BLK], DT, kind="ExternalInput").ap()
    wkT = nc.dram_tensor("wkT", [HID, BLK], DT, kind="ExternalInput").ap()
    wvT = nc.dram_tensor("wvT", [HID, BLK], DT, kind="ExternalInput").ap()
    woT = nc.dram_tensor("woT", [BLK, HID], DT, kind="ExternalInput").ap()
    cos2 = nc.dram_tensor("cos2", [128, S], fp32, kind="ExternalInput").ap()
    sin2s = nc.dram_tensor("sin2s", [128, S], fp32, kind="ExternalInput").ap()
    if not causal:
        maskT = nc.dram_tensor("maskT", [S, S], mybir.dt.bfloat16,
                               kind="ExternalInput").ap()
    out = nc.dram_tensor("out", [S, HID], fp32, kind="ExternalOutput").ap()

    with tile.TileContext(nc) as tc:
        with tc.tile_pool(name="qk_sb", bufs=1) as qk_pool:
            # qT/kT: [qdim-pair-row 128, pair 4, token 2048]
            qT_sb = qk_pool.tile([128, NPR, S], SDT)
            kT_sb = qk_pool.tile([128, NPR, S], SDT)

            # ---------------- phase 1a: Q and K projections + RoPE -------
            with tc.tile_pool(name="consts", bufs=1) as cpool, \
                 tc.tile_pool(name="wqk", bufs=1) as wpool, \
                 tc.tile_pool(name="xa", bufs=2) as xa, \
                 tc.tile_pool(name="ropetmp", bufs=2) as rt, \
                 tc.tile_pool(name="psqk", bufs=1, space="PSUM") as psqk:
                cos_sb = cpool.tile([128, S], fp32)
                sin_sb = cpool.tile([128, S], fp32)
                nc.sync.dma_start(out=cos_sb, in_=cos2)
                nc.sync.dma_start(out=sin_sb, in_=sin2s)
                wq_sb = wpool.tile([128, 16, BLK], DT)
                wk_sb = wpool.tile([128, 16, BLK], DT)
                nc.sync.dma_start(out=wq_sb, in_=wqT.rearrange("(k p) m -> p k m", p=128))
                nc.sync.dma_start(out=wk_sb, in_=wkT.rearrange("(k p) m -> p k m", p=128))

                for n in range(NCH):
                    sl = slice(n * CH, (n + 1) * CH)
                    x_ch = xa.tile([128, 16, CH], DT, name=f"x1a_{n}", tag="x")
                    nc.sync.dma_start(out=x_ch, in_=xT[:, sl].rearrange("(k p) t -> p k t", p=128))
                    for w_sb, dst in ((wq_sb, qT_sb), (wk_sb, kT_sb)):
                        wn = "q" if dst is qT_sb else "k"
                        for m in range(NPR):
                            ps = psqk.tile([128, CH], fp32, name=f"ps{wn}{m}_{n}", tag=f"ps{wn}{m}")
                            for k in range(16):
                                nc.tensor.matmul(ps,
                                                 lhsT=mm(w_sb[:, k, m * 128:(m + 1) * 128]),
                                                 rhs=mm(x_ch[:, k, :]),
                                                 start=(k == 0), stop=(k == 15))
                            # RoPE fused evict: dst[:, m, sl] =
                            #   ps * cos + shift32(ps) * sin_signed
                            t1 = rt.tile([128, CH], fp32, name=f"t1{wn}{m}_{n}", tag="t1")
                            t2 = rt.tile([128, CH], fp32, name=f"t2{wn}{m}_{n}", tag="t2")
                            nc.vector.tensor_mul(t1, ps, cos_sb[:, sl])
                            for (d, s_) in ((0, 32), (32, 0), (64, 96), (96, 64)):
                                nc.vector.tensor_mul(t2[d:d + 32, :],
                                                     ps[s_:s_ + 32, :],
                                                     sin_sb[d:d + 32, sl])
                            nc.vector.tensor_add(dst[:, m, sl], t1, t2)

            # ---------------- phase 1b: V projection ---------------------
            with tc.tile_pool(name="v_sb_pool", bufs=1) as v_pool:
                # v: [tk-in-tile 128, tile 16, head-major 8*65] with ones col
                v_sb = v_pool.tile([128, NT, HEADS_PER_CORE * 65], SDT)
                with tc.tile_pool(name="wv", bufs=1) as wvp, \
                     tc.tile_pool(name="xb", bufs=2) as xb, \
                     tc.tile_pool(name="psv", bufs=2, space="PSUM") as psv:
                    # ones columns (written once; memset per 65-stride col)
                    ones_ap = v_sb.rearrange("p t (h c) -> p t h c", c=65)[:, :, :, 64:65]
                    nc.vector.memset(ones_ap, 1.0)
                    wv_sb = wvp.tile([128, 16, BLK], DT)
                    nc.sync.dma_start(out=wv_sb, in_=wvT.rearrange("(k p) m -> p k m", p=128))
                    for n in range(NCH):
                        sl = slice(n * CH, (n + 1) * CH)
                        x_ch = xb.tile([128, 16, CH], DT, name=f"x1b_{n}", tag="x")
                        nc.sync.dma_start(out=x_ch, in_=xT[:, sl].rearrange("(k p) t -> p k t", p=128))
                        for s_ in range(CH // 128):
                            t16 = (n * CH) // 128 + s_
                            ps = psv.tile([128, BLK], fp32, name=f"psv{t16}", tag="psv")
                            for k in range(16):
                                nc.tensor.matmul(ps,
                                                 lhsT=mm(x_ch[:, k, s_ * 128:(s_ + 1) * 128]),
                                                 rhs=mm(wv_sb[:, k, :]),
                                                 start=(k == 0), stop=(k == 15))
                            # evict to v_sb columns h*65 .. h*65+64
                            dst = v_sb[:, t16, :].rearrange("p (h c) -> p h c", c=65)[:, :, 0:64]
                            nc.scalar.copy(dst, ps.rearrange("p (h c) -> p h c", c=64))

                # ---------------- phase 2: attention ---------------------
                with tc.tile_pool(name="attn_pool", bufs=1) as apool:
                    attn_sb = apool.tile([128, NPR, S], SDT)
                    with tc.tile_pool(name="ppool", bufs=3) as ppool, \
                         tc.tile_pool(name="npool", bufs=2) as npool, \
                         tc.tile_pool(name="mpool", bufs=2) as mpool, \
                         tc.tile_pool(name="dpool", bufs=2, space="DRAM") as dpool, \
                         tc.tile_pool(name="pss", bufs=2, space="PSUM") as pss, \
                         tc.tile_pool(name="psav", bufs=2, space="PSUM") as psav:

                        def attend(pr, j, mask_col):
                            av = []
                            n_i = 4 * j + 4 if causal else NT
                            for half in range(2):
                                av.append(psav.tile([65, 512], fp32,
                                                    name=f"av{half}_{pr}_{j}", tag=f"av{half}"))
                            for i in range(n_i):
                                p_t = []
                                for half in range(2):
                                    h = 2 * pr + half
                                    r0 = 64 * half
                                    s_ps = pss.tile([128, 512], fp32,
                                                    name=f"s{half}_{pr}_{j}_{i}", tag=f"s{half}")
                                    nc.tensor.matmul(
                                        s_ps,
                                        lhsT=mm(kT_sb[r0:r0 + 64, pr, i * 128:(i + 1) * 128]),
                                        rhs=mm(qT_sb[r0:r0 + 64, pr, j * 512:(j + 1) * 512]),
                                        start=True, stop=True,
                                        tile_position=(r0, 0))
                                    p = ppool.tile([128, 512], SDT,
                                                   name=f"p{half}_{pr}_{j}_{i}", tag=f"p{half}")
                                    if causal:
                                        nc.scalar.activation(p, s_ps,
                                                             mybir.ActivationFunctionType.Exp,
                                                             scale=0.125)
                                        if i >= 4 * j:
                                            # keep iff tq - tk >= 0:
                                            #   -part + free + (512j - 128i) >= 0
                                            nc.gpsimd.affine_select(
                                                out=p, in_=p,
                                                compare_op=mybir.AluOpType.is_ge,
                                                fill=0.0,
                                                base=512 * j - 128 * i,
                                                pattern=[[1, 512]],
                                                channel_multiplier=-1)
                                    else:
                                        tmp = ppool.tile([128, 512], fp32,
                                                         name=f"pt{half}_{pr}_{j}_{i}", tag=f"pt{half}")
                                        nc.vector.scalar_tensor_tensor(
                                            out=tmp, in0=s_ps, scalar=0.125,
                                            in1=mask_col[:, i, :],
                                            op0=mybir.AluOpType.mult,
                                            op1=mybir.AluOpType.add)
                                        nc.scalar.activation(p, tmp,
                                                             mybir.ActivationFunctionType.Exp)
                                    p_t.append(p)
                                for half in range(2):
                                    h = 2 * pr + half
                                    nc.tensor.matmul(
                                        av[half],
                                        lhsT=mm(v_sb[:, i, 65 * h:65 * h + 65]),
                                        rhs=mm(p_t[half]),
                                        start=(i == 0), stop=(i == n_i - 1))
                            # normalize: rows 0..63 / row 64
                            for half in range(2):
                                r0 = 64 * half
                                rec = npool.tile([1, 512], fp32,
                                                 name=f"rec{half}_{pr}_{j}", tag=f"rec{half}")
                                nc.vector.reciprocal(rec, av[half][64:65, :])
                                # partition-broadcast via DRAM bounce (step-0
                                # partition APs are only legal on DRAM)
                                rec_d = dpool.tile([1, 512], fp32,
                                                   name=f"rd{half}_{pr}_{j}", tag=f"rd{half}")
                                nc.sync.dma_start(out=rec_d, in_=rec)
                                bc = npool.tile([64, 512], fp32,
                                                name=f"bc{half}_{pr}_{j}", tag=f"bc{half}")
                                bc_src = bass.AP(tensor=rec_d.tensor, offset=rec_d.offset,
                                                 ap=[[0, 64]] + [list(p) for p in rec_d.ap[1:]])
                                nc.sync.dma_start(out=bc, in_=bc_src)
                                nc.vector.tensor_mul(
                                    attn_sb[r0:r0 + 64, pr, j * 512:(j + 1) * 512],
                                    av[half][0:64, :], bc)

                        if causal:
                            for pr in range(NPR):
                                for j in range(4):
                                    attend(pr, j, None)
                        else:
                            for j in range(4):
                                mask_col = mpool.tile([128, NT, 512], mybir.dt.bfloat16,
                                                      name=f"mc{j}", tag="mc")
                                nc.sync.dma_start(
                                    out=mask_col,
                                    in_=maskT[:, j * 512:(j + 1) * 512]
                                    .rearrange("(i p) t -> p i t", p=128))
                                for pr in range(NPR):
                                    attend(pr, j, mask_col)

                    # ---------------- phase 3: output projection ---------
                    with tc.tile_pool(name="wo", bufs=1) as wop, \
                         tc.tile_pool(name="outstage", bufs=3) as ostage, \
                         tc.tile_pool(name="pso", bufs=2, space="PSUM") as pso:
                        wo_sb = wop.tile([128, NPR, HID], DT)
                        nc.sync.dma_start(out=wo_sb, in_=woT.rearrange("(r p) o -> p r o", p=128))
                        for t16 in range(NT):
                            o_ps = [pso.tile([128, 512], fp32, name=f"o{t16}_{no}", tag=f"o{no}")
                                    for no in range(4)]
                            for pr in range(NPR):
                                for no in range(4):
                                    nc.tensor.matmul(
                                        o_ps[no],
                                        lhsT=mm(attn_sb[:, pr, t16 * 128:(t16 + 1) * 128]),
                                        rhs=mm(wo_sb[:, pr, no * 512:(no + 1) * 512]),
                                        start=(pr == 0), stop=(pr == NPR - 1))
                            for no in range(4):
                                o_sb = ostage.tile([128, 512], fp32,
                                                   name=f"os{t16}_{no}", tag="os")
                                nc.scalar.copy(o_sb, o_ps[no])
                                nc.sync.dma_start(
                                    out=out[t16 * 128:(t16 + 1) * 128,
                                            no * 512:(no + 1) * 512],
                                    in_=o_sb)
    nc.compile()
    return nc


def _get_program(variant, dtype):
    key = (variant, dtype)
    if key not in _PROGRAMS:
        _PROGRAMS[key] = build_program(variant, dtype)
    return _PROGRAMS[key]


def _np_dt(dtype):
    return ml_dtypes.bfloat16 if dtype == "bf16" else np.float32


def make_in_maps(hidden_states, attention_mask, position_ids, Wq, Wk, Wv, Wo,
                 variant, dtype):
    npdt = _np_dt(dtype)
    cos, sin = _rope_cache()
    in_maps = []
    for c in range(N_CORES):
        b = c // (N_CORES // 2)
        hb = c % (N_CORES // 2)
        rs = slice(hb * BLK, (hb + 1) * BLK)
        pos = np.asarray(position_ids[b]).astype(np.int64)
        cos_b = cos[pos].T.astype(np.float32)     # [64, S]
        sin_b = sin[pos].T.astype(np.float32)
        sin_s = np.concatenate([-sin_b[:32], sin_b[32:]], axis=0)
        m = {
            "xT": np.ascontiguousarray(hidden_states[b].T).astype(npdt),
            "wqT": np.ascontiguousarray(Wq[rs].T).astype(npdt),
            "wkT": np.ascontiguousarray(Wk[rs].T).astype(npdt),
            "wvT": np.ascontiguousarray(Wv[rs].T).astype(npdt),
            "woT": np.ascontiguousarray(Wo[:, rs].T).astype(npdt),
            "cos2": np.ascontiguousarray(np.concatenate([cos_b, cos_b], axis=0)),
            "sin2s": np.ascontiguousarray(np.concatenate([sin_s, sin_s], axis=0)),
        }
        if variant == "general":
            m["maskT"] = np.ascontiguousarray(
                attention_mask[b, 0].T).astype(ml_dtypes.bfloat16)
        in_maps.append(m)
    return in_maps


def detect_causal(attention_mask):
    am = np.asarray(attention_mask)
    if am.shape != (2, 1, S, S):
        return False
    neg = np.float32(np.finfo(np.float32).min)
    canonical = np.where(np.tril(np.ones((S, S), dtype=bool)), np.float32(0.0), neg)
    return bool(np.array_equal(am[0, 0], canonical) and
                np.array_equal(am[1, 0], canonical))


DTYPE = "bf16"


def kernel(hidden_states, attention_mask, position_ids, Wq, Wk, Wv, Wo):
    hidden_states = np.asarray(hidden_states, dtype=np.float32)
    attention_mask = np.asarray(attention_mask, dtype=np.float32)
    Wq, Wk, Wv, Wo = (np.asarray(w, dtype=np.float32) for w in (Wq, Wk, Wv, Wo))

    variant = "causal" if detect_causal(attention_mask) else "general"
    nc = _get_program(variant, DTYPE)
    in_maps = make_in_maps(hidden_states, attention_mask, position_ids,
                           Wq, Wk, Wv, Wo, variant, DTYPE)

    from concourse import bass2jax
    results = bass2jax.run_bass_via_pjrt(nc, in_maps, n_cores=N_CORES)

    out = np.zeros((2, S, HID), dtype=np.float64)
    for c in range(N_CORES):
        b = c // (N_CORES // 2)
        out[b] += results[c]["out"].astype(np.float64)
    return out.astype(np.float32)
